# revision 44
# baseline (speedup 1.0000x reference)
"""Trainium2 Bass kernel for the dual-branch spatial-reduction attention module.

Sharding (8 NeuronCores): branch-parallel (cores 0-3 -> branch 0, cores 4-7 ->
branch 1); within a branch quad, query-token-parallel for the attention (each
core owns 1024 of the 4096 query tokens) and kv-position-parallel for the
spatial-reduction conv + LayerNorm (each core computes 256 of the 1024 kv
positions, then the normalized transposed activations are AllGather'd across
the quad). The host does layout prep (transposes, bf16 casts, weight folding,
conv patch gathering) and the final TokenExchange between branches.

Per-core program (all matmuls bf16 with fp32 PSUM accumulation):
  - conv = 4 shifted matmuls over host-pregathered 2x2 patches (token-major,
    only this core's 2 of 8 position chunks) + bias via a K=1 ones-matmul;
    LayerNorm fused on the PSUM output (bn_stats/bn_aggr, vector reciprocal);
    gamma/beta folded into Wkv/bias on the host; xn transposed to
    channel-major via bf16 DMA-XBAR 128x128 tiles
  - AllGather of the xnT quarter across the branch quad (gpsimd collective,
    local DRAM bounce); q proj (softmax scale folded into Wq) overlaps it
  - k proj channel-major / v proj token-major on the gathered xnT, biases
    fused into the PSUM->SBUF copies (per-partition add for k, gpsimd
    partition-broadcast row add for v); v gets an appended ones column so
    the softmax denominator falls out of the attn@v matmul (row 64)
  - attention: qk for a head pair packed into PE row groups 0-63/64-127
    (K=64 each, concurrent on hardware); exp on ScalarE over 2-bank PSUM
    tiles -> bf16 probability tiles; attn@v accumulates [65, 512] PSUM
    (row 64 = softmax denominator); softmax without max subtraction
    (logits are tiny by construction); per-(head, n) normalization via
    gpsimd partition_broadcast of the reciprocal row + one DVE multiply
  - out proj token-major per n-chunk right after its attention completes;
    mask applied via per-partition tensor_scalar_mul
"""

import sys

sys.path.insert(0, "/opt/trn_rl_repo")

import numpy as np
import ml_dtypes

BF16 = ml_dtypes.bfloat16

NUM_HEADS = 8
SR = 2
LN_EPS = 1e-5
MASK_THRESHOLD = 0.02
B, N, C = 1, 4096, 512
H = W = 64
M = N // (SR * SR)  # 1024 kv positions
HD = C // NUM_HEADS  # 64
NQ = N // 4  # 1024 query tokens per core
N_CORES = 8

_compiled = None


def _build():
    import concourse.bass as bass
    import concourse.tile as tile
    from concourse import bacc, mybir

    f32 = mybir.dt.float32
    bf16 = mybir.dt.bfloat16

    nc = bacc.Bacc("TRN2", target_bir_lowering=False, debug=False,
                   num_devices=N_CORES)

    # ---- DRAM I/O ----
    MQ = M // 4  # 256 kv positions owned per core
    xp_d = nc.dram_tensor("xp", [4, C, MQ], bf16, kind="ExternalInput")
    xq_d = nc.dram_tensor("xqT", [C, NQ], bf16, kind="ExternalInput")
    wq_d = nc.dram_tensor("wq", [C, C], bf16, kind="ExternalInput")
    wsr_d = nc.dram_tensor("wsr", [4, C, C], bf16, kind="ExternalInput")
    bsr_d = nc.dram_tensor("bsr_r", [1, C], bf16, kind="ExternalInput")
    wk_d = nc.dram_tensor("wk", [C, C], bf16, kind="ExternalInput")
    bkc_d = nc.dram_tensor("bk_col", [128, 4], f32, kind="ExternalInput")
    wv_d = nc.dram_tensor("wv", [C, C], bf16, kind="ExternalInput")
    bv_d = nc.dram_tensor("bv_r", [1, C], f32, kind="ExternalInput")
    wp_d = nc.dram_tensor("wp", [C, C], bf16, kind="ExternalInput")
    bp_d = nc.dram_tensor("bp_r", [1, C], bf16, kind="ExternalInput")
    mask_d = nc.dram_tensor("mask_s", [128, 8], f32, kind="ExternalInput")
    push_d = nc.dram_tensor("xn_push", [128, 4 * MQ], bf16)
    gath_d = nc.dram_tensor("xn_gath", [4, 128, 4 * MQ], bf16)
    out_d = nc.dram_tensor("out", [NQ, C], f32, kind="ExternalOutput")

    P = 128
    CC = C // P  # 4 channel chunks
    MC = M // P  # 8 kv-position chunks
    N2 = NQ // 512  # 2 query free-dim chunks
    HP = NUM_HEADS // 2  # 4 head pairs

    from contextlib import ExitStack
    with tile.TileContext(nc) as tc, ExitStack() as ctx:
        consts = ctx.enter_context(tc.tile_pool(name="consts", bufs=1))
        psA = ctx.enter_context(tc.tile_pool(name="psA", bufs=2, space="PSUM"))
        psQK = ctx.enter_context(tc.tile_pool(name="psQK", bufs=2, space="PSUM"))
        psAV = ctx.enter_context(tc.tile_pool(name="psAV", bufs=2, space="PSUM"))
        ptp = ctx.enter_context(tc.tile_pool(name="ptp", bufs=22))
        xrp = ctx.enter_context(tc.tile_pool(name="xrp", bufs=2))
        xnp = ctx.enter_context(tc.tile_pool(name="xnp", bufs=6))
        stats = ctx.enter_context(tc.tile_pool(name="stats", bufs=4))
        outs = ctx.enter_context(tc.tile_pool(name="outs", bufs=4))

        def load_cpn(dst, src, eng=None):
            (eng or nc.sync).dma_start(
                out=dst, in_=src.rearrange("(cc p) n -> p cc n", p=P))

        # ---- input DMAs: conv inputs fine-grained in consumption order,
        # wsr on the sync queue and xp on the scalar queue so the conv can
        # start streaming after the first (cc, ij) chunk lands ----
        bsr_sb = consts.tile([1, C], bf16)
        nc.sync.dma_start(out=bsr_sb, in_=bsr_d.ap())
        wsr_sb = consts.tile([P, 4, CC, C], bf16)
        nc.sync.dma_start(
            out=wsr_sb, in_=wsr_d.ap().rearrange("ij (cc p) n -> p ij cc n", p=P))
        xp_sb = consts.tile([P, 4, CC, MQ], bf16)
        nc.scalar.dma_start(
            out=xp_sb, in_=xp_d.ap().rearrange("ij (cc p) n -> p ij cc n", p=P))
        wq_sb = consts.tile([P, CC, C], bf16)
        load_cpn(wq_sb, wq_d)
        xq_sb = consts.tile([P, CC, NQ], bf16)
        load_cpn(xq_sb, xq_d)
        wk_sb = consts.tile([P, CC, C], bf16)
        load_cpn(wk_sb, wk_d)
        wv_sb = consts.tile([P, CC, C], bf16)
        load_cpn(wv_sb, wv_d)
        bkc_sb = consts.tile([P, 4], f32)
        nc.sync.dma_start(out=bkc_sb, in_=bkc_d.ap())
        bv_sb = consts.tile([1, C], f32)
        nc.sync.dma_start(out=bv_sb, in_=bv_d.ap())
        wp_sb = consts.tile([P, CC, C], bf16)
        load_cpn(wp_sb, wp_d)
        bp_sb = consts.tile([1, C], bf16)
        nc.sync.dma_start(out=bp_sb, in_=bp_d.ap())
        mask_sb = consts.tile([P, 8], f32)
        nc.sync.dma_start(out=mask_sb, in_=mask_d.ap())

        ones128 = consts.tile([1, P], bf16)
        nc.vector.memset(ones128, 1.0)
        ones512 = consts.tile([1, C], bf16)
        nc.vector.memset(ones512, 1.0)
        eps_sb = consts.tile([P, 1], f32)
        nc.vector.memset(eps_sb, LN_EPS)

        # PE warm-up: ~4us of dummy matmuls during the input-DMA wait so the
        # HAM clock gate is released before the conv starts
        warm_ps = psA.tile([1, 512], f32, tag="psA")
        for w in range(20):
            nc.tensor.matmul(warm_ps, ones128[0:1, 0:1], ones512,
                             start=(w == 0), stop=(w == 19))

        # broadcast v-bias row to a full partition tile (gpsimd, off hot engines)
        bvB = consts.tile([P, C], f32)
        nc.gpsimd.partition_broadcast(bvB, bv_sb)

        qT_sb = consts.tile([P, CC, NQ], bf16)
        kT_sb = consts.tile([P, CC, M], bf16)
        v_sb = consts.tile([P, MC, NUM_HEADS, HD + 1], bf16)
        xnT_sb = consts.tile([P, CC, MQ], bf16)
        xnTF_sb = consts.tile([P, CC, M], bf16)
        attn_sb = consts.tile([P, CC, NQ], bf16)

        nc.vector.memset(v_sb[:, :, :, HD:HD + 1], 1.0)

        # ---- sharded conv/LN: this core owns 2 of the 8 mc chunks ----
        for i in range(2):
            ps = psA.tile([P, 512], f32, tag="psA")
            first = True
            for cc in range(CC):
                for ij in range(4):
                    nc.tensor.matmul(
                        ps, xp_sb[:, ij, cc, i * P:(i + 1) * P],
                        wsr_sb[:, ij, cc, :], start=first, stop=False)
                    first = False
            nc.tensor.matmul(ps, ones128, bsr_sb, start=False, stop=True)
            st6 = stats.tile([P, 6], f32, tag="st6")
            nc.vector.bn_stats(out=st6, in_=ps)
            mv = stats.tile([P, 2], f32, tag="mv")
            nc.vector.bn_aggr(out=mv, in_=st6)
            std = stats.tile([P, 1], f32, tag="std")
            nc.scalar.activation(
                out=std, in_=mv[:, 1:2],
                func=mybir.ActivationFunctionType.Sqrt, bias=eps_sb, scale=1.0)
            rstd = stats.tile([P, 1], f32, tag="rstd")
            nc.vector.reciprocal(out=rstd, in_=std)
            xn = xnp.tile([P, 512], bf16, tag="xn")
            nc.vector.tensor_scalar(
                out=xn, in0=ps, scalar1=mv[:, 0:1], scalar2=rstd,
                op0=mybir.AluOpType.subtract, op1=mybir.AluOpType.mult)
            for cc in range(CC):
                nc.scalar.dma_start(
                    out=xnT_sb[:, cc, i * P:(i + 1) * P],
                    in_=xn[:, cc * P:(cc + 1) * P],
                    transpose=True)

        # ---- AllGather the normalized, transposed quarter across the quad ----
        nc.scalar.dma_start(out=push_d.ap(), in_=xnT_sb)
        nc.gpsimd.collective_compute(
            "AllGather", mybir.AluOpType.bypass,
            replica_groups=[[0, 1, 2, 3], [4, 5, 6, 7]],
            ins=[push_d.ap()], outs=[gath_d.ap()])
        for s in range(4):
            nc.sync.dma_start(
                out=xnTF_sb[:, :, s * MQ:(s + 1) * MQ],
                in_=gath_d[s].rearrange("p (a b) -> p a b", a=CC))

        # ---- q projection (channel-major): qT[cq, n] ----
        for n2 in range(N2):
            for cq in range(CC):
                ps = psA.tile([P, 512], f32, tag="psA")
                for cc in range(CC):
                    nc.tensor.matmul(
                        ps, wq_sb[:, cc, cq * P:(cq + 1) * P],
                        xq_sb[:, cc, n2 * 512:(n2 + 1) * 512],
                        start=(cc == 0), stop=(cc == CC - 1))
                nc.vector.tensor_copy(
                    out=qT_sb[:, cq, n2 * 512:(n2 + 1) * 512], in_=ps)

        # ---- v proj on gathered xnT first (attn@v needs it), then k proj
        # per head-pair chunk interleaved with that pair's attention so
        # ScalarE exp starts as early as possible ----
        def vproj_all():
            for mc in range(MC):
                ps = psA.tile([P, 512], f32, tag="psA")
                for cc in range(CC):
                    nc.tensor.matmul(
                        ps, xnTF_sb[:, cc, mc * P:(mc + 1) * P], wv_sb[:, cc, :],
                        start=(cc == 0), stop=(cc == CC - 1))
                nc.vector.tensor_tensor(
                    out=v_sb[:, mc, :, 0:HD], in0=ps, in1=bvB,
                    op=mybir.AluOpType.add)

        def k_proj(ck):
            for m2 in range(M // 512):
                ps = psA.tile([P, 512], f32, tag="psA")
                for cc in range(CC):
                    nc.tensor.matmul(
                        ps, wk_sb[:, cc, ck * P:(ck + 1) * P],
                        xnTF_sb[:, cc, m2 * 512:(m2 + 1) * 512],
                        start=(cc == 0), stop=(cc == CC - 1))
                nc.vector.tensor_scalar_add(
                    out=kT_sb[:, ck, m2 * 512:(m2 + 1) * 512],
                    in0=ps, scalar1=bkc_sb[:, ck:ck + 1])

        def qk_exp(n2, hp, mc_list=None):
            pts = []
            for mc in (mc_list if mc_list is not None else range(MC)):
                qk = psQK.tile([P, 1024], f32, tag="psQK")
                nc.tensor.matmul(
                    qk[:, 0:512],
                    kT_sb[0:HD, hp, mc * P:(mc + 1) * P],
                    qT_sb[0:HD, hp, n2 * 512:(n2 + 1) * 512],
                    start=True, stop=True)
                nc.tensor.matmul(
                    qk[:, 512:1024],
                    kT_sb[HD:P, hp, mc * P:(mc + 1) * P],
                    qT_sb[HD:P, hp, n2 * 512:(n2 + 1) * 512],
                    start=True, stop=True)
                pt = ptp.tile([P, 1024], bf16, tag="pt")
                nc.scalar.activation(
                    out=pt, in_=qk, func=mybir.ActivationFunctionType.Exp)
                pts.append(pt)
            return pts

        def finish_pair(n2, hp, av_e, av_o):
            for par, av in ((0, av_e), (1, av_o)):
                avf = outs.tile([HD + 1, 512], f32, tag="avf")
                nc.vector.tensor_copy(out=avf, in_=av)
                rs = stats.tile([1, 512], f32, tag="rs")
                nc.vector.reciprocal(out=rs, in_=avf[HD:HD + 1, :])
                rsb = stats.tile([HD, 512], f32, tag="rsb")
                nc.gpsimd.partition_broadcast(rsb, rs)
                nc.vector.tensor_mul(
                    out=attn_sb[HD * par:HD * (par + 1), hp,
                                n2 * 512:(n2 + 1) * 512],
                    in0=avf[0:HD, :], in1=rsb)

        def av_pair(n2, hp, pts, pool=None):
            pool = pool or psAV
            av_e = pool.tile([HD + 1, 512], f32, tag="psA" if pool is psA else "psAV")
            av_o = pool.tile([HD + 1, 512], f32, tag="psA" if pool is psA else "psAV")
            for mc, pt in enumerate(pts):
                nc.tensor.matmul(
                    av_e, v_sb[:, mc, 2 * hp, :], pt[:, 0:512],
                    start=(mc == 0), stop=(mc == MC - 1))
                nc.tensor.matmul(
                    av_o, v_sb[:, mc, 2 * hp + 1, :], pt[:, 512:1024],
                    start=(mc == 0), stop=(mc == MC - 1))
            finish_pair(n2, hp, av_e, av_o)

        def attn_pair(n2, hp, pool=None):
            pool = pool or psAV
            av_e = pool.tile([HD + 1, 512], f32, tag="psA" if pool is psA else "psAV")
            av_o = pool.tile([HD + 1, 512], f32, tag="psA" if pool is psA else "psAV")
            for mc in range(MC):
                pt = qk_exp(n2, hp, [mc])[0]
                nc.tensor.matmul(
                    av_e, v_sb[:, mc, 2 * hp, :], pt[:, 0:512],
                    start=(mc == 0), stop=(mc == MC - 1))
                nc.tensor.matmul(
                    av_o, v_sb[:, mc, 2 * hp + 1, :], pt[:, 512:1024],
                    start=(mc == 0), stop=(mc == MC - 1))
            finish_pair(n2, hp, av_e, av_o)

        def out_proj(t):
            ps = psA.tile([P, 512], f32, tag="psA")
            for cc in range(CC):
                nc.tensor.matmul(
                    ps, attn_sb[:, cc, t * P:(t + 1) * P], wp_sb[:, cc, :],
                    start=(cc == 0), stop=False)
            nc.tensor.matmul(ps, ones128, bp_sb, start=False, stop=True)
            ot = outs.tile([P, C], f32, tag="ot")
            nc.vector.tensor_scalar_mul(
                out=ot, in0=ps, scalar1=mask_sb[:, t:t + 1])
            nc.scalar.dma_start(out=out_d[t * P:(t + 1) * P, :], in_=ot)

        k_proj(0)
        k_proj(1)
        k_proj(2)
        pts00 = qk_exp(0, 0)   # feeds ScalarE while v proj runs on the PE
        pts01 = qk_exp(0, 1)
        vproj_all()
        av_pair(0, 0, pts00)
        pts02 = qk_exp(0, 2)
        av_pair(0, 1, pts01)
        av_pair(0, 2, pts02)
        k_proj(3)
        attn_pair(0, 3)
        # n2=0 projections interleave with the n2=1 attention pairs so the
        # PE-side proj work never starves ScalarE of qk tiles
        for hp in range(HP - 1):
            attn_pair(1, hp)
            out_proj(hp)
        # last pair borrows the idle psA banks so its attn@v overlaps the
        # previous pair instead of waiting on the psAV rotation
        attn_pair(1, HP - 1, pool=psA)
        out_proj(3)
        for t in range(4, 8):
            out_proj(t)

    nc.compile()
    return nc


def _prep_inputs(x0, x1, mask0, mask1, Wq, Wkv, Wsr, bsr, gamma, beta, Wp, bp):
    """Host-side layout prep -> per-core in_maps."""
    scale = HD ** (-0.5)
    wq = (Wq * scale).astype(BF16)
    # conv weights: Wsr[co, ci, i, j] -> per (i,j) lhs [ci, co]
    wsr = np.stack([Wsr[:, :, ij // 2, ij % 2].T.copy() for ij in range(4)])
    wsr = wsr.astype(BF16)
    bsr_r = bsr.reshape(1, C).astype(np.float32)
    # fold LN gamma/beta into Wkv
    Wkv_f = gamma[:, None] * Wkv
    bkv = beta @ Wkv
    wk = Wkv_f[:, :C].astype(BF16)
    bk_col = np.ascontiguousarray(
        bkv[:C].reshape(4, 128).T).astype(np.float32)
    wv = Wkv_f[:, C:].astype(BF16)
    bv_r = bkv[C:].reshape(1, C).astype(np.float32)
    wp = Wp.astype(BF16)
    bp_r = bp.reshape(1, C).astype(BF16)

    shared = dict(wq=wq, wsr=wsr, bsr_r=bsr_r, wk=wk, bk_col=bk_col,
                  wv=wv, bv_r=bv_r, wp=wp, bp_r=bp_r)

    xT = [np.ascontiguousarray(x[0].T).astype(BF16) for x in (x0, x1)]
    # patch-major gather for the conv: xp[ij][c, oh*32+ow] = xT[c, 128*oh+64*i+2*ow+j]
    xp = []
    for b in range(2):
        v = xT[b].reshape(C, 32, 2, 32, 2)
        xp.append(np.stack([
            np.ascontiguousarray(v[:, :, ij // 2, :, ij % 2].reshape(C, M))
            for ij in range(4)]))
    masks = (mask0, mask1)
    in_maps = []
    MQ = M // 4
    for core in range(N_CORES):
        b, s = core // 4, core % 4
        m = dict(shared)
        m["xp"] = np.ascontiguousarray(xp[b][:, :, s * MQ:(s + 1) * MQ])
        m["xqT"] = np.ascontiguousarray(xT[b][:, s * NQ:(s + 1) * NQ])
        msk = masks[b][0, s * NQ:(s + 1) * NQ]
        m["mask_s"] = np.ascontiguousarray(
            msk.reshape(NQ // 128, 128).T).astype(np.float32)
        in_maps.append(m)
    return in_maps


def kernel(x0, x1, mask0, mask1, Wq, Wkv, Wsr, bsr, gamma, beta, Wp, bp,
           H=64, W=64, _trace=False):
    global _compiled
    x0 = np.asarray(x0, np.float32)
    x1 = np.asarray(x1, np.float32)
    mask0 = np.asarray(mask0, np.float32)
    mask1 = np.asarray(mask1, np.float32)
    assert x0.shape == (B, N, C) and int(H) == 64 and int(W) == 64

    from concourse.bass_utils import run_bass_kernel_spmd

    if _compiled is None:
        _compiled = _build()
    nc = _compiled

    in_maps = _prep_inputs(
        x0, x1, mask0, mask1,
        np.asarray(Wq, np.float32), np.asarray(Wkv, np.float32),
        np.asarray(Wsr, np.float32), np.asarray(bsr, np.float32),
        np.asarray(gamma, np.float32), np.asarray(beta, np.float32),
        np.asarray(Wp, np.float32), np.asarray(bp, np.float32))

    kw = {}
    if _trace:
        kw = dict(trace=True, trace_cores=[0])
    try:
        res = run_bass_kernel_spmd(nc, in_maps, list(range(N_CORES)), **kw)
    except ModuleNotFoundError:
        # NTFF profile hook unavailable in this environment -> run untraced
        res = run_bass_kernel_spmd(nc, in_maps, list(range(N_CORES)))

    o0 = np.concatenate([res.results[i]["out"] for i in range(4)], axis=0)
    o1 = np.concatenate([res.results[i]["out"] for i in range(4, 8)], axis=0)
    keep0 = (mask0[0] >= MASK_THRESHOLD)[:, None]
    keep1 = (mask1[0] >= MASK_THRESHOLD)[:, None]
    y0 = np.where(keep0, o0, o1)[None]
    y1 = np.where(keep1, o1, o0)[None]
    out = np.stack([y0, y1]).astype(np.float32)
    if _trace:
        kernel._last_result = res
    return out


kernel._last_result = None
